# revision 18
# baseline (speedup 1.0000x reference)
import sys

sys.path.insert(0, "/opt/trn_rl_repo")

from contextlib import ExitStack

import numpy as np

import concourse.bacc as bacc
import concourse.mybir as mybir
from concourse import tile
from concourse.bass_utils import run_bass_kernel_spmd

F32 = mybir.dt.float32
AL = mybir.AluOpType
AF = mybir.ActivationFunctionType

C = 256
H = W = 64
NC = 8  # cores / batch shards


# ---------------------------------------------------------------- host prep
def host_prep(inp):
    """Rearrange all weights into [partition, free] layouts matching SBUF tiles."""
    d = {}
    f = np.float32

    # conditioning nets (dsc1, dsc2)
    for i, pre in ((0, "dsc1"), (1, "dsc2")):
        w1 = np.asarray(inp[f"{pre}_w1"], f)  # [64, 256]
        b1 = np.asarray(inp[f"{pre}_b1"], f)  # [64]
        w2 = np.asarray(inp[f"{pre}_w2"], f)  # [2304, 64]
        b2 = np.asarray(inp[f"{pre}_b2"], f)  # [2304]
        # lhsT for gm matmul: [k_local, chunk, m]; fold the 1/(H*W) mean here
        d[f"w1T{i}"] = np.ascontiguousarray(
            (w1.T / (H * W)).reshape(2, 128, 64).transpose(1, 0, 2)
        ).reshape(128, 128)
        d[f"b1_{i}"] = b1.reshape(64, 1).copy()
        # lhsT for wts matmul: [j, chunk, k, c_local]; fold gelu's 0.5 here
        d[f"w2r{i}"] = np.ascontiguousarray(
            (0.5 * w2).reshape(2, 128, 9, 64).transpose(3, 0, 2, 1)
        ).reshape(64, 2304)
        d[f"b2r{i}"] = np.ascontiguousarray(
            b2.reshape(2, 128, 9).transpose(1, 0, 2)
        ).reshape(128, 18)

    # channel_align 1x1: [k_local, kc, mc, m]
    aw = np.asarray(inp["align_w"], f)[:, :, 0, 0]  # [256, 512]
    d["alignw"] = np.ascontiguousarray(
        aw.reshape(2, 128, 4, 128).transpose(3, 2, 0, 1)
    ).reshape(128, 1024)
    d["alignb"] = np.ascontiguousarray(
        np.asarray(inp["align_b"], f).reshape(2, 128).T
    )  # [128, 2]

    # up conv1 3x3 C->4C with pixel-shuffle phase reorder:
    # new channel (p, g) -> orig channel 4g + p   (p = 2r+s)
    uw = np.asarray(inp["up_w1"], f)  # [1024, 256, 3, 3]
    a = uw.reshape(256, 4, 2, 128, 9)  # [g, p, kc, k_local, tap]
    a = a.reshape(2, 128, 4, 2, 128, 9)  # [mcin, m, p, kc, k_local, tap]
    d["up1w"] = np.ascontiguousarray(a.transpose(4, 2, 0, 5, 3, 1)).reshape(
        128, 4 * 2 * 9 * 2 * 128
    )  # [k_local, (p, mcin, tap, kc, m)]
    ub = np.asarray(inp["up_b1"], f)
    d["up1b"] = np.ascontiguousarray(
        ub.reshape(2, 128, 4).transpose(1, 2, 0)
    ).reshape(128, 8)  # [m, (p, mcin)]

    # up conv2 1x1 C->C/2 (per-phase): [k_local, kc, m]
    u2 = np.asarray(inp["up_w2"], f)[:, :, 0, 0]  # [128, 256]
    d["up2w"] = np.ascontiguousarray(
        u2.reshape(128, 2, 128).transpose(2, 1, 0)
    ).reshape(128, 256)
    d["up2b"] = np.asarray(inp["up_b2"], f).reshape(128, 1).copy()

    # ---- polyphase re_enhance ----
    # phase/tap -> (in-phase, sub-shift) mapping
    def split(v):  # v = r + dy - 1
        rp = v % 2
        return rp, (v - rp) // 2

    r1w = np.asarray(inp["re_w1"], f)  # [32, 128, 3, 3]
    keymap = {}
    for p in range(4):
        r, s = p // 2, p % 2
        for dy in range(3):
            for dx in range(3):
                rp, qy = split(r + dy - 1)
                sp, qx = split(s + dx - 1)
                keymap.setdefault((2 * rp + sp, qy, qx), []).append((p, dy, dx))
    keys = sorted(keymap.keys(), key=lambda k: (k[1] != 0 or k[2] != 0, k))
    re1_keys = keys  # list of (p_in, qy, qx); all-(0,0) shifts first
    re1w = np.zeros((128, 16, 128), f)
    for ki, key in enumerate(keys):
        for (p, dy, dx) in keymap[key]:
            re1w[:, ki, p * 32 : (p + 1) * 32] = r1w[:, :, dy, dx].T
    d["re1w"] = re1w.reshape(128, 2048)
    d["re1b"] = np.tile(np.asarray(inp["re_b1"], f), 4).reshape(128, 1)

    r2w = np.asarray(inp["re_w2"], f)  # [128, 32, 3, 3]
    re2_q = []  # per out-phase list of shifts, (0,0) first
    re2w = np.zeros((128, 4, 4, 128), f)
    for p in range(4):
        r, s = p // 2, p % 2
        qys = sorted({split(r + dy - 1)[1] for dy in range(3)}, key=lambda q: q != 0)
        qxs = sorted({split(s + dx - 1)[1] for dx in range(3)}, key=lambda q: q != 0)
        qs = [(qy, qx) for qy in qys for qx in qxs]
        qs.sort(key=lambda q: q != (0, 0))
        re2_q.append(qs)
        for qi, (qy, qx) in enumerate(qs):
            for pp in range(4):
                rp, sp = pp // 2, pp % 2
                dy = 2 * qy + rp - r + 1
                dx = 2 * qx + sp - s + 1
                if 0 <= dy < 3 and 0 <= dx < 3:
                    re2w[pp * 32 : (pp + 1) * 32, p, qi, :] = r2w[:, :, dy, dx].T
    d["re2w"] = re2w.reshape(128, 2048)
    d["re2b"] = np.asarray(inp["re_b2"], f).reshape(128, 1).copy()

    return d, re1_keys, re2_q


RE1_KEYS = None
RE2_Q = None


def _mapping():
    global RE1_KEYS, RE2_Q
    if RE1_KEYS is None:
        zeros = {k: np.zeros(v) for k, v in [
            ("dsc1_w1", (64, 256)), ("dsc1_b1", (64,)), ("dsc1_w2", (2304, 64)),
            ("dsc1_b2", (2304,)), ("dsc2_w1", (64, 256)), ("dsc2_b1", (64,)),
            ("dsc2_w2", (2304, 64)), ("dsc2_b2", (2304,)),
            ("align_w", (256, 512, 1, 1)), ("align_b", (256,)),
            ("up_w1", (1024, 256, 3, 3)), ("up_b1", (1024,)),
            ("up_w2", (128, 256, 1, 1)), ("up_b2", (128,)),
            ("re_w1", (32, 128, 3, 3)), ("re_b1", (32,)),
            ("re_w2", (128, 32, 3, 3)), ("re_b2", (128,)),
        ]}
        _, RE1_KEYS, RE2_Q = host_prep(zeros)
    return RE1_KEYS, RE2_Q


# ---------------------------------------------------------------- bass build
def img(ap):
    return ap.rearrange("p (y x) -> p y x", y=64)


def shift_views(tile_ap, sy, sx):
    """(src, dst) views of a flat [128,4096] image for shift (sy,sx)."""
    y0 = max(0, -sy)
    r = 64 - abs(sy)
    x0 = max(0, -sx)
    xw = 64 - abs(sx)
    v = img(tile_ap)
    src = v[:, y0 + sy : y0 + sy + r, x0 + sx : x0 + sx + xw]
    dst = v[:, y0 : y0 + r, x0 : x0 + xw]
    return src, dst


def pimg(ap):
    """View of a column-padded [128, 64*66] image (zero cols at x=0 and x=65)."""
    return ap.rearrange("p (y x) -> p y x", x=66)


def mm_views(src_ap, psum_ap, sy, sx, n):
    """(psum_out, rhs) for 8-row chunk n of a shifted conv tap on a col-padded
    source. Row range restricted by sy; columns handled by the zero pad.
    psum out is a contiguous 2D region."""
    y0 = max(0, -sy)
    y1 = 64 + min(0, -sy)
    r0 = max(8 * n, y0)
    r1 = min(8 * n + 8, y1)
    if r1 <= r0:
        return None, None
    v = pimg(src_ap)
    rhs = v[:, r0 + sy : r1 + sy, 1 + sx : 65 + sx]
    out = psum_ap[:, (r0 - 8 * n) * 64 : (r1 - 8 * n) * 64]
    return out, rhs


def build():
    re1_keys, re2_q = _mapping()
    nc = bacc.Bacc(trn_type="TRN2", target_bir_lowering=False, debug=False)

    x_d = [nc.dram_tensor(n, [256, 4096], F32, kind="ExternalInput") for n in ("x1", "x2")]
    wd = {}
    for name, shape in [
        ("w1T0", [128, 128]), ("w1T1", [128, 128]),
        ("b1_0", [64, 1]), ("b1_1", [64, 1]),
        ("w2r0", [64, 2304]), ("w2r1", [64, 2304]),
        ("b2r0", [128, 18]), ("b2r1", [128, 18]),
        ("alignw", [128, 1024]), ("alignb", [128, 2]),
        ("up1w", [128, 18432]), ("up1b", [128, 8]),
        ("up2w", [128, 256]), ("up2b", [128, 1]),
        ("re1w", [128, 2048]), ("re1b", [128, 1]),
        ("re2w", [128, 2048]), ("re2b", [128, 1]),
    ]:
        wd[name] = nc.dram_tensor(name, shape, F32, kind="ExternalInput")
    out_d = nc.dram_tensor("out", [128, 16384], F32, kind="ExternalOutput")

    with tile.TileContext(nc) as tc, ExitStack() as ctx:
        wpool = ctx.enter_context(tc.tile_pool(name="w", bufs=1))
        u1pool = ctx.enter_context(tc.tile_pool(name="u1w", bufs=2))
        big = ctx.enter_context(tc.tile_pool(name="big", bufs=7))
        stg = ctx.enter_context(tc.tile_pool(name="stg", bufs=2))
        bnd = ctx.enter_context(tc.tile_pool(name="bnd", bufs=2))
        tiny = ctx.enter_context(tc.tile_pool(name="tiny", bufs=8))
        ps = ctx.enter_context(tc.tile_pool(name="ps", bufs=5, space="PSUM"))
        psc = ctx.enter_context(tc.tile_pool(name="psc", bufs=1, space="PSUM"))

        # persistent small weights
        wt = {}
        for name in ("w1T0", "w1T1", "b1_0", "b1_1", "b2r0", "b2r1",
                     "alignw", "alignb", "up2w", "up2b",
                     "re1w", "re1b", "re2w", "re2b", "up1b"):
            t = wpool.tile(list(wd[name].shape), F32, tag=name)
            nc.sync.dma_start(t[:], wd[name].ap())
            wt[name] = t

        # x in
        xin = []  # [input][chunk] -> big tile
        for i in range(2):
            pair = []
            for c in range(2):
                t = big.tile([128, 4096], F32, tag="big")
                nc.sync.dma_start(t[:], x_d[i].ap()[c * 128 : (c + 1) * 128, :])
                pair.append(t)
            xin.append(pair)

        # streamed conditioning w2r (one dsc at a time)
        w2rt = {}

        def load_w2r(d):
            if d not in w2rt:
                t = u1pool.tile([64, 2304], F32, tag="u1w")
                nc.sync.dma_start(t[:], wd[f"w2r{d}"].ap())
                w2rt[d] = t
            return w2rt[d]

        # ---------------- dsc stage ----------------
        def conditioning(d, gms):
            """gms: list of two [128,1] sum tiles -> wts [chunk][128,9], w4p [chunk][128,1]."""
            pg = psc.tile([64, 1], F32, tag="psc_g")
            for c in range(2):
                nc.tensor.matmul(pg[:], wt[f"w1T{d}"][:, c * 64 : (c + 1) * 64],
                                 gms[c][:], start=(c == 0), stop=(c == 1))
            u = tiny.tile([64, 1], F32, tag="u")
            nc.scalar.activation(u[:], pg[:], AF.Identity, bias=wt[f"b1_{d}"][:])
            sq = tiny.tile([64, 1], F32, tag="sq")
            nc.scalar.activation(sq[:], u[:], AF.Square)
            v3 = tiny.tile([64, 1], F32, tag="v3")
            nc.vector.scalar_tensor_tensor(v3[:], sq[:], 0.044715, u[:], AL.mult, AL.mult)
            w_ = tiny.tile([64, 1], F32, tag="w_")
            nc.vector.tensor_tensor(w_[:], v3[:], u[:], AL.add)
            th = tiny.tile([64, 1], F32, tag="th")
            nc.scalar.activation(th[:], w_[:], AF.Tanh, scale=0.7978845608028654)
            hv = tiny.tile([64, 1], F32, tag="hv")
            nc.vector.scalar_tensor_tensor(hv[:], th[:], 1.0, u[:], AL.add, AL.mult)

            w2r = load_w2r(d)
            wts_l, w4p_l = [], []
            for c in range(2):
                pw = psc.tile([128, 9], F32, tag="psc_w")
                for k in range(9):
                    nc.tensor.matmul(pw[:, k : k + 1],
                                     w2r[:, (c * 9 + k) * 128 : (c * 9 + k + 1) * 128],
                                     hv[:], start=True, stop=True)
                raw = tiny.tile([128, 9], F32, tag="raw")
                for k in range(9):
                    nc.scalar.activation(raw[:, k : k + 1], pw[:, k : k + 1], AF.Identity,
                                         bias=wt[f"b2r{d}"][:, c * 9 + k : c * 9 + k + 1])
                mx = tiny.tile([128, 1], F32, tag="mx")
                nc.vector.tensor_reduce(mx[:], raw[:], axis=mybir.AxisListType.X, op=AL.max)
                ngm = tiny.tile([128, 1], F32, tag="ngm")
                nc.vector.tensor_scalar_mul(ngm[:], mx[:], -1.0)
                ex = tiny.tile([128, 9], F32, tag="ex")
                ssum = tiny.tile([128, 1], F32, tag="ssum")
                nc.scalar.activation(ex[:], raw[:], AF.Exp, bias=ngm[:], accum_out=ssum[:])
                rec = tiny.tile([128, 1], F32, tag="rec")
                nc.vector.reciprocal(rec[:], ssum[:])
                wts = tiny.tile([128, 9], F32, tag="wts")
                nc.vector.tensor_scalar_mul(wts[:], ex[:], rec[:])
                w4p = tiny.tile([128, 1], F32, tag="w4p")
                nc.vector.tensor_scalar_add(w4p[:], wts[:, 4:5], 1.0)
                wts_l.append(wts)
                w4p_l.append(w4p)
            return wts_l, w4p_l

        def depthwise(src, dst, wts, w4p):
            """dst = sum_k wts[:,k] * shift_k(src) + src   (flat [128,4096] tiles)."""
            nc.vector.tensor_scalar(dst[:], src[:], w4p[:], None, AL.mult)
            for k in (0, 1, 2, 3, 5, 6, 7, 8):
                sy, sx = k // 3 - 1, k % 3 - 1
                sv, dv = shift_views(src[:], sy, sx)
                _, acc = shift_views(dst[:], sy, sx)
                nc.vector.scalar_tensor_tensor(acc, sv, wts[:, k : k + 1], acc,
                                               AL.mult, AL.add)

        y = [[None, None], [None, None]]  # dyn_block outputs
        mid = [[None, None], [None, None]]
        gms2 = [[None, None], [None, None]]
        for i in range(2):
            gms1 = []
            for c in range(2):
                g = tiny.tile([128, 1], F32, tag="gms")
                nc.vector.tensor_reduce(g[:], xin[i][c][:], axis=mybir.AxisListType.X, op=AL.add)
                gms1.append(g)
            wts1, w4p1 = conditioning(0, gms1)
            for c in range(2):
                m = big.tile([128, 4096], F32, tag="big")
                depthwise(xin[i][c], m, wts1[c], w4p1[c])
                g2 = tiny.tile([128, 1], F32, tag="gms")
                nc.scalar.activation(m[:], m[:], AF.Relu, accum_out=g2[:])
                mid[i][c] = m
                gms2[i][c] = g2
        for i in range(2):
            wts2, w4p2 = conditioning(1, gms2[i])
            for c in range(2):
                yt = big.tile([128, 4096], F32, tag="big")
                depthwise(mid[i][c], yt, wts2[c], w4p2[c])
                y[i][c] = yt

        # ---------------- align 1x1 (2C -> C) ----------------
        fused = []
        for mc in range(2):
            f = big.tile([128, 4224], F32, tag="big")
            nc.gpsimd.memset(f[:], 0.0)
            fused.append(f)
        for mc in range(2):
            for n in range(8):
                p = ps.tile([128, 512], F32, tag="ps")
                for kc in range(4):
                    rhs = y[kc // 2][kc % 2][:, n * 512 : (n + 1) * 512]
                    nc.tensor.matmul(
                        p[:], wt["alignw"][:, (kc * 2 + mc) * 128 : (kc * 2 + mc + 1) * 128],
                        rhs, start=(kc == 0), stop=(kc == 3))
                nc.scalar.activation(pimg(fused[mc][:])[:, n * 8 : (n + 1) * 8, 1:65],
                                     p[:].rearrange("p (y x) -> p y x", y=8),
                                     AF.Identity, bias=wt["alignb"][:, mc : mc + 1])

        # ---------------- up1 (3x3 C->4C, phase-ordered) + up2 (1x1) ----------------
        taps = [(1, 1)] + [(dy, dx) for dy in range(3) for dx in range(3) if (dy, dx) != (1, 1)]
        up2p = []
        for p4 in range(4):
            t = big.tile([128, 4224], F32, tag="big")
            nc.gpsimd.memset(t[:], 0.0)
            up2p.append(t)
        for p4 in range(4):
            u1t = []
            for mcin in range(2):
                t = u1pool.tile([128, 2304], F32, tag="u1w")
                off = (p4 * 2 + mcin) * 2304
                nc.sync.dma_start(t[:], wd["up1w"].ap()[:, off : off + 2304])
                u1t.append(t)
            stage = []
            for mcin in range(2):
                st = stg.tile([128, 4096], F32, tag="stg")
                stage.append(st)
            for n in range(8):
                for mcin in range(2):
                    p = ps.tile([128, 512], F32, tag="ps")
                    first = True
                    for (dy, dx) in taps:
                        sy, sx = dy - 1, dx - 1
                        for kc in range(2):
                            o, rhs = mm_views(fused[kc][:], p[:], sy, sx, n)
                            if o is None:
                                continue
                            lhsT = u1t[mcin][:, ((dy * 3 + dx) * 2 + kc) * 128 :
                                             ((dy * 3 + dx) * 2 + kc + 1) * 128]
                            nc.tensor.matmul(o, lhsT, rhs, start=first,
                                             stop=((dy, dx) == taps[-1] and kc == 1),
                                             skip_group_check=True)
                            first = False
                    nc.scalar.activation(stage[mcin][:, n * 512 : (n + 1) * 512], p[:],
                                         AF.Identity,
                                         bias=wt["up1b"][:, p4 * 2 + mcin : p4 * 2 + mcin + 1])
                p2 = ps.tile([128, 512], F32, tag="ps")
                for kc in range(2):
                    nc.tensor.matmul(p2[:], wt["up2w"][:, kc * 128 : (kc + 1) * 128],
                                     stage[kc][:, n * 512 : (n + 1) * 512],
                                     start=(kc == 0), stop=(kc == 1))
                nc.scalar.activation(pimg(up2p[p4][:])[:, n * 8 : (n + 1) * 8, 1:65],
                                     p2[:].rearrange("p (y x) -> p y x", y=8),
                                     AF.Identity, bias=wt["up2b"][:])

        # ---------------- re1 (polyphase 3x3, M-packed) ----------------
        re1t = big.tile([128, 4224], F32, tag="big")
        nc.gpsimd.memset(re1t[:], 0.0)
        for n in range(8):
            p = ps.tile([128, 512], F32, tag="ps")
            for ki, (pin, qy, qx) in enumerate(re1_keys):
                o, rhs = mm_views(up2p[pin][:], p[:], qy, qx, n)
                if o is None:
                    continue
                nc.tensor.matmul(o, wt["re1w"][:, ki * 128 : (ki + 1) * 128], rhs,
                                 start=(ki == 0), stop=(ki == len(re1_keys) - 1),
                                 skip_group_check=True)
            nc.scalar.activation(pimg(re1t[:])[:, n * 8 : (n + 1) * 8, 1:65],
                                 p[:].rearrange("p (y x) -> p y x", y=8),
                                 AF.Relu, bias=wt["re1b"][:])

        # ---------------- re2 (polyphase 3x3) + residual + interleave + out ----------------
        for n in range(8):
            pss = []
            for p4 in range(4):
                p = ps.tile([128, 512], F32, tag="ps")
                for qi, (qy, qx) in enumerate(re2_q[p4]):
                    o, rhs = mm_views(re1t[:], p[:], qy, qx, n)
                    if o is None:
                        continue
                    nc.tensor.matmul(o, wt["re2w"][:, (p4 * 4 + qi) * 128 :
                                                   (p4 * 4 + qi + 1) * 128], rhs,
                                     start=(qi == 0), stop=(qi == len(re2_q[p4]) - 1),
                                     skip_group_check=True)
                pss.append(p)
            for hb in range(2):  # half-bands of 8 output rows (4 phase rows)
                band = bnd.tile([128, 1024], F32, tag="bnd")
                bv = band[:].rearrange("p (y r x s) -> p y r x s", y=4, r=2, s=2)
                for p4 in range(4):
                    r, s = p4 // 2, p4 % 2
                    y0 = n * 8 + hb * 4
                    up_v = pimg(up2p[p4][:])[:, y0 : y0 + 4, 1:65]
                    nc.vector.scalar_tensor_tensor(
                        bv[:, :, r, :, s],
                        pss[p4][:, hb * 256 : (hb + 1) * 256].rearrange(
                            "p (y x) -> p y x", y=4),
                        wt["re2b"][:], up_v, AL.add, AL.add)
                nc.sync.dma_start(
                    out_d.ap()[:, (2 * n + hb) * 1024 : (2 * n + hb + 1) * 1024],
                    band[:])

    nc.compile()
    return nc


_NC = None


def _get_nc():
    global _NC
    if _NC is None:
        _NC = build()
    return _NC


def make_in_maps(inputs):
    w, _, _ = host_prep(inputs)
    x1 = np.ascontiguousarray(np.asarray(inputs["x1"], np.float32).reshape(NC, 256, 4096))
    x2 = np.ascontiguousarray(np.asarray(inputs["x2"], np.float32).reshape(NC, 256, 4096))
    in_maps = []
    for i in range(NC):
        m = {"x1": x1[i], "x2": x2[i]}
        m.update(w)
        in_maps.append(m)
    return in_maps


def kernel(**inputs):
    nc = _get_nc()
    in_maps = make_in_maps(inputs)
    res = run_bass_kernel_spmd(nc, in_maps, core_ids=list(range(NC)))
    out = np.stack([res.results[i]["out"].reshape(128, 128, 128) for i in range(NC)])
    return out.astype(np.float32)


# revision 19
# speedup vs baseline: 60.0659x; 60.0659x over previous
import sys

sys.path.insert(0, "/opt/trn_rl_repo")

from contextlib import ExitStack

import numpy as np

import concourse.bacc as bacc
import concourse.mybir as mybir
from concourse import tile
from concourse.bass_utils import run_bass_kernel_spmd

F32 = mybir.dt.float32
AL = mybir.AluOpType
AF = mybir.ActivationFunctionType

C = 256
H = W = 64
NC = 8  # cores / batch shards


# ---------------------------------------------------------------- host prep
def host_prep(inp):
    """Rearrange all weights into [partition, free] layouts matching SBUF tiles."""
    d = {}
    f = np.float32

    # conditioning nets (dsc1, dsc2)
    for i, pre in ((0, "dsc1"), (1, "dsc2")):
        w1 = np.asarray(inp[f"{pre}_w1"], f)  # [64, 256]
        b1 = np.asarray(inp[f"{pre}_b1"], f)  # [64]
        w2 = np.asarray(inp[f"{pre}_w2"], f)  # [2304, 64]
        b2 = np.asarray(inp[f"{pre}_b2"], f)  # [2304]
        # lhsT for gm matmul: [k_local, chunk, m]; fold the 1/(H*W) mean here
        d[f"w1T{i}"] = np.ascontiguousarray(
            (w1.T / (H * W)).reshape(2, 128, 64).transpose(1, 0, 2)
        ).reshape(128, 128)
        d[f"b1_{i}"] = b1.reshape(64, 1).copy()
        # lhsT for wts matmul: [j, chunk, k, c_local]; fold gelu's 0.5 here
        d[f"w2r{i}"] = np.ascontiguousarray(
            (0.5 * w2).reshape(2, 128, 9, 64).transpose(3, 0, 2, 1)
        ).reshape(64, 2304)
        d[f"b2r{i}"] = np.ascontiguousarray(
            b2.reshape(2, 128, 9).transpose(1, 0, 2)
        ).reshape(128, 18)

    # channel_align 1x1: [k_local, kc, mc, m]
    aw = np.asarray(inp["align_w"], f)[:, :, 0, 0]  # [256, 512]
    d["alignw"] = np.ascontiguousarray(
        aw.reshape(2, 128, 4, 128).transpose(3, 2, 0, 1)
    ).reshape(128, 1024)
    d["alignb"] = np.ascontiguousarray(
        np.asarray(inp["align_b"], f).reshape(2, 128).T
    )  # [128, 2]

    # up conv1 3x3 C->4C with pixel-shuffle phase reorder:
    # new channel (p, g) -> orig channel 4g + p   (p = 2r+s)
    uw = np.asarray(inp["up_w1"], f)  # [1024, 256, 3, 3]
    a = uw.reshape(256, 4, 2, 128, 9)  # [g, p, kc, k_local, tap]
    a = a.reshape(2, 128, 4, 2, 128, 9)  # [mcin, m, p, kc, k_local, tap]
    d["up1w"] = np.ascontiguousarray(a.transpose(4, 2, 0, 5, 3, 1)).reshape(
        128, 4 * 2 * 9 * 2 * 128
    )  # [k_local, (p, mcin, tap, kc, m)]
    ub = np.asarray(inp["up_b1"], f)
    d["up1b"] = np.ascontiguousarray(
        ub.reshape(2, 128, 4).transpose(1, 2, 0)
    ).reshape(128, 8)  # [m, (p, mcin)]

    # up conv2 1x1 C->C/2 (per-phase): [k_local, kc, m]
    u2 = np.asarray(inp["up_w2"], f)[:, :, 0, 0]  # [128, 256]
    d["up2w"] = np.ascontiguousarray(
        u2.reshape(128, 2, 128).transpose(2, 1, 0)
    ).reshape(128, 256)
    d["up2b"] = np.asarray(inp["up_b2"], f).reshape(128, 1).copy()

    # ---- polyphase re_enhance ----
    # phase/tap -> (in-phase, sub-shift) mapping
    def split(v):  # v = r + dy - 1
        rp = v % 2
        return rp, (v - rp) // 2

    r1w = np.asarray(inp["re_w1"], f)  # [32, 128, 3, 3]
    keymap = {}
    for p in range(4):
        r, s = p // 2, p % 2
        for dy in range(3):
            for dx in range(3):
                rp, qy = split(r + dy - 1)
                sp, qx = split(s + dx - 1)
                keymap.setdefault((2 * rp + sp, qy, qx), []).append((p, dy, dx))
    keys = sorted(keymap.keys(), key=lambda k: (k[1] != 0 or k[2] != 0, k))
    re1_keys = keys  # list of (p_in, qy, qx); all-(0,0) shifts first
    re1w = np.zeros((128, 16, 128), f)
    for ki, key in enumerate(keys):
        for (p, dy, dx) in keymap[key]:
            re1w[:, ki, p * 32 : (p + 1) * 32] = r1w[:, :, dy, dx].T
    d["re1w"] = re1w.reshape(128, 2048)
    d["re1b"] = np.tile(np.asarray(inp["re_b1"], f), 4).reshape(128, 1)

    r2w = np.asarray(inp["re_w2"], f)  # [128, 32, 3, 3]
    re2_q = []  # per out-phase list of shifts, (0,0) first
    re2w = np.zeros((128, 4, 4, 128), f)
    for p in range(4):
        r, s = p // 2, p % 2
        qys = sorted({split(r + dy - 1)[1] for dy in range(3)}, key=lambda q: q != 0)
        qxs = sorted({split(s + dx - 1)[1] for dx in range(3)}, key=lambda q: q != 0)
        qs = [(qy, qx) for qy in qys for qx in qxs]
        qs.sort(key=lambda q: q != (0, 0))
        re2_q.append(qs)
        for qi, (qy, qx) in enumerate(qs):
            for pp in range(4):
                rp, sp = pp // 2, pp % 2
                dy = 2 * qy + rp - r + 1
                dx = 2 * qx + sp - s + 1
                if 0 <= dy < 3 and 0 <= dx < 3:
                    re2w[pp * 32 : (pp + 1) * 32, p, qi, :] = r2w[:, :, dy, dx].T
    d["re2w"] = re2w.reshape(128, 2048)
    d["re2b"] = np.asarray(inp["re_b2"], f).reshape(128, 1).copy()

    return d, re1_keys, re2_q


RE1_KEYS = None
RE2_Q = None


def _mapping():
    global RE1_KEYS, RE2_Q
    if RE1_KEYS is None:
        zeros = {k: np.zeros(v) for k, v in [
            ("dsc1_w1", (64, 256)), ("dsc1_b1", (64,)), ("dsc1_w2", (2304, 64)),
            ("dsc1_b2", (2304,)), ("dsc2_w1", (64, 256)), ("dsc2_b1", (64,)),
            ("dsc2_w2", (2304, 64)), ("dsc2_b2", (2304,)),
            ("align_w", (256, 512, 1, 1)), ("align_b", (256,)),
            ("up_w1", (1024, 256, 3, 3)), ("up_b1", (1024,)),
            ("up_w2", (128, 256, 1, 1)), ("up_b2", (128,)),
            ("re_w1", (32, 128, 3, 3)), ("re_b1", (32,)),
            ("re_w2", (128, 32, 3, 3)), ("re_b2", (128,)),
        ]}
        _, RE1_KEYS, RE2_Q = host_prep(zeros)
    return RE1_KEYS, RE2_Q


# ---------------------------------------------------------------- bass build
def img(ap):
    return ap.rearrange("p (y x) -> p y x", y=64)


def shift_views(tile_ap, sy, sx):
    """(src, dst) views of a flat [128,4096] image for shift (sy,sx)."""
    y0 = max(0, -sy)
    r = 64 - abs(sy)
    x0 = max(0, -sx)
    xw = 64 - abs(sx)
    v = img(tile_ap)
    src = v[:, y0 + sy : y0 + sy + r, x0 + sx : x0 + sx + xw]
    dst = v[:, y0 : y0 + r, x0 : x0 + xw]
    return src, dst


def pimg(ap):
    """View of a column-padded [128, 64*66] image (zero cols at x=0 and x=65)."""
    return ap.rearrange("p (y x) -> p y x", x=66)


def mm_views(src_ap, psum_ap, sy, sx, n):
    """(psum_out, rhs) for 8-row chunk n of a shifted conv tap on a col-padded
    source. Row range restricted by sy; columns handled by the zero pad.
    psum out is a contiguous 2D region."""
    y0 = max(0, -sy)
    y1 = 64 + min(0, -sy)
    r0 = max(8 * n, y0)
    r1 = min(8 * n + 8, y1)
    if r1 <= r0:
        return None, None
    v = pimg(src_ap)
    rhs = v[:, r0 + sy : r1 + sy, 1 + sx : 65 + sx]
    out = psum_ap[:, (r0 - 8 * n) * 64 : (r1 - 8 * n) * 64]
    return out, rhs


def build():
    re1_keys, re2_q = _mapping()
    nc = bacc.Bacc(trn_type="TRN2", target_bir_lowering=False, debug=False)

    x_d = [nc.dram_tensor(n, [256, 4096], F32, kind="ExternalInput") for n in ("x1", "x2")]
    wd = {}
    for name, shape in [
        ("w1T0", [128, 128]), ("w1T1", [128, 128]),
        ("b1_0", [64, 1]), ("b1_1", [64, 1]),
        ("w2r0", [64, 2304]), ("w2r1", [64, 2304]),
        ("b2r0", [128, 18]), ("b2r1", [128, 18]),
        ("alignw", [128, 1024]), ("alignb", [128, 2]),
        ("up1w", [128, 18432]), ("up1b", [128, 8]),
        ("up2w", [128, 256]), ("up2b", [128, 1]),
        ("re1w", [128, 2048]), ("re1b", [128, 1]),
        ("re2w", [128, 2048]), ("re2b", [128, 1]),
    ]:
        wd[name] = nc.dram_tensor(name, shape, F32, kind="ExternalInput")
    out_d = nc.dram_tensor("out", [128, 16384], F32, kind="ExternalOutput")

    with tile.TileContext(nc) as tc, ExitStack() as ctx:
        wpool = ctx.enter_context(tc.tile_pool(name="w", bufs=1))
        u1pool = ctx.enter_context(tc.tile_pool(name="u1w", bufs=2))
        big = ctx.enter_context(tc.tile_pool(name="big", bufs=7))
        stg = ctx.enter_context(tc.tile_pool(name="stg", bufs=2))
        bnd = ctx.enter_context(tc.tile_pool(name="bnd", bufs=2))
        tiny = ctx.enter_context(tc.tile_pool(name="tiny", bufs=6))
        ps = ctx.enter_context(tc.tile_pool(name="ps", bufs=5, space="PSUM"))
        psc = ctx.enter_context(tc.tile_pool(name="psc", bufs=1, space="PSUM"))

        # persistent small weights
        wt = {}
        for name in ("w1T0", "w1T1", "b1_0", "b1_1", "b2r0", "b2r1",
                     "alignw", "alignb", "up2w", "up2b",
                     "re1w", "re1b", "re2w", "re2b", "up1b"):
            t = wpool.tile(list(wd[name].shape), F32, tag=name)
            nc.sync.dma_start(t[:], wd[name].ap())
            wt[name] = t

        # x in
        xin = []  # [input][chunk] -> big tile
        for i in range(2):
            pair = []
            for c in range(2):
                t = big.tile([128, 4096], F32, tag="big")
                nc.sync.dma_start(t[:], x_d[i].ap()[c * 128 : (c + 1) * 128, :])
                pair.append(t)
            xin.append(pair)

        # streamed conditioning w2r (one dsc at a time)
        w2rt = {}

        def load_w2r(d):
            if d not in w2rt:
                t = u1pool.tile([64, 2304], F32, tag="u1w")
                nc.sync.dma_start(t[:], wd[f"w2r{d}"].ap())
                w2rt[d] = t
            return w2rt[d]

        # ---------------- dsc stage ----------------
        def conditioning(d, gms):
            """gms: list of two [128,1] sum tiles -> wts [chunk][128,9], w4p [chunk][128,1]."""
            pg = psc.tile([64, 1], F32, tag="psc_g")
            for c in range(2):
                nc.tensor.matmul(pg[:], wt[f"w1T{d}"][:, c * 64 : (c + 1) * 64],
                                 gms[c][:], start=(c == 0), stop=(c == 1))
            u = tiny.tile([64, 1], F32, tag="u")
            nc.scalar.activation(u[:], pg[:], AF.Identity, bias=wt[f"b1_{d}"][:])
            sq = tiny.tile([64, 1], F32, tag="sq")
            nc.scalar.activation(sq[:], u[:], AF.Square)
            v3 = tiny.tile([64, 1], F32, tag="v3")
            nc.vector.scalar_tensor_tensor(v3[:], sq[:], 0.044715, u[:], AL.mult, AL.mult)
            w_ = tiny.tile([64, 1], F32, tag="w_")
            nc.vector.tensor_tensor(w_[:], v3[:], u[:], AL.add)
            th = tiny.tile([64, 1], F32, tag="th")
            nc.scalar.activation(th[:], w_[:], AF.Tanh, scale=0.7978845608028654)
            hv = tiny.tile([64, 1], F32, tag="hv")
            nc.vector.scalar_tensor_tensor(hv[:], th[:], 1.0, u[:], AL.add, AL.mult)

            w2r = load_w2r(d)
            wts_l, w4p_l = [], []
            for c in range(2):
                pw = psc.tile([128, 9], F32, tag="psc_w")
                for k in range(9):
                    nc.tensor.matmul(pw[:, k : k + 1],
                                     w2r[:, (c * 9 + k) * 128 : (c * 9 + k + 1) * 128],
                                     hv[:], start=True, stop=True)
                raw = tiny.tile([128, 9], F32, tag="raw")
                for k in range(9):
                    nc.scalar.activation(raw[:, k : k + 1], pw[:, k : k + 1], AF.Identity,
                                         bias=wt[f"b2r{d}"][:, c * 9 + k : c * 9 + k + 1])
                mx = tiny.tile([128, 1], F32, tag="mx")
                nc.vector.tensor_reduce(mx[:], raw[:], axis=mybir.AxisListType.X, op=AL.max)
                ngm = tiny.tile([128, 1], F32, tag="ngm")
                nc.vector.tensor_scalar_mul(ngm[:], mx[:], -1.0)
                ex = tiny.tile([128, 9], F32, tag="ex")
                ssum = tiny.tile([128, 1], F32, tag="ssum")
                nc.scalar.activation(ex[:], raw[:], AF.Exp, bias=ngm[:], accum_out=ssum[:])
                rec = tiny.tile([128, 1], F32, tag="rec")
                nc.vector.reciprocal(rec[:], ssum[:])
                wts = tiny.tile([128, 9], F32, tag="wts")
                nc.vector.tensor_scalar_mul(wts[:], ex[:], rec[:])
                w4p = tiny.tile([128, 1], F32, tag="w4p")
                nc.vector.tensor_scalar_add(w4p[:], wts[:, 4:5], 1.0)
                wts_l.append(wts)
                w4p_l.append(w4p)
            return wts_l, w4p_l

        def depthwise(src, dst, wts, w4p):
            """dst = sum_k wts[:,k] * shift_k(src) + src   (flat [128,4096] tiles)."""
            nc.vector.tensor_scalar(dst[:], src[:], w4p[:], None, AL.mult)
            for k in (0, 1, 2, 3, 5, 6, 7, 8):
                sy, sx = k // 3 - 1, k % 3 - 1
                sv, dv = shift_views(src[:], sy, sx)
                _, acc = shift_views(dst[:], sy, sx)
                nc.vector.scalar_tensor_tensor(acc, sv, wts[:, k : k + 1], acc,
                                               AL.mult, AL.add)

        y = [[None, None], [None, None]]  # dyn_block outputs
        mid = [[None, None], [None, None]]
        gms2 = [[None, None], [None, None]]
        for i in range(2):
            gms1 = []
            for c in range(2):
                g = tiny.tile([128, 1], F32, tag="gms")
                nc.vector.tensor_reduce(g[:], xin[i][c][:], axis=mybir.AxisListType.X, op=AL.add)
                gms1.append(g)
            wts1, w4p1 = conditioning(0, gms1)
            for c in range(2):
                m = big.tile([128, 4096], F32, tag="big")
                depthwise(xin[i][c], m, wts1[c], w4p1[c])
                g2 = tiny.tile([128, 1], F32, tag="gms")
                nc.scalar.activation(m[:], m[:], AF.Relu, accum_out=g2[:])
                mid[i][c] = m
                gms2[i][c] = g2
        for i in range(2):
            wts2, w4p2 = conditioning(1, gms2[i])
            for c in range(2):
                yt = big.tile([128, 4096], F32, tag="big")
                depthwise(mid[i][c], yt, wts2[c], w4p2[c])
                y[i][c] = yt

        # ---------------- align 1x1 (2C -> C) ----------------
        fused = []
        for mc in range(2):
            f = big.tile([128, 4224], F32, tag="big")
            nc.gpsimd.memset(f[:], 0.0)
            fused.append(f)
        for mc in range(2):
            for n in range(8):
                p = ps.tile([128, 512], F32, tag="ps")
                for kc in range(4):
                    rhs = y[kc // 2][kc % 2][:, n * 512 : (n + 1) * 512]
                    nc.tensor.matmul(
                        p[:], wt["alignw"][:, (kc * 2 + mc) * 128 : (kc * 2 + mc + 1) * 128],
                        rhs, start=(kc == 0), stop=(kc == 3))
                nc.scalar.activation(pimg(fused[mc][:])[:, n * 8 : (n + 1) * 8, 1:65],
                                     p[:].rearrange("p (y x) -> p y x", y=8),
                                     AF.Identity, bias=wt["alignb"][:, mc : mc + 1])

        # ---------------- up1 (3x3 C->4C, phase-ordered) + up2 (1x1) ----------------
        taps = [(1, 1)] + [(dy, dx) for dy in range(3) for dx in range(3) if (dy, dx) != (1, 1)]
        up2p = []
        for p4 in range(4):
            t = big.tile([128, 4224], F32, tag="big")
            nc.gpsimd.memset(t[:], 0.0)
            up2p.append(t)
        for p4 in range(4):
            u1t = []
            for mcin in range(2):
                t = u1pool.tile([128, 2304], F32, tag="u1w")
                off = (p4 * 2 + mcin) * 2304
                nc.sync.dma_start(t[:], wd["up1w"].ap()[:, off : off + 2304])
                u1t.append(t)
            stage = []
            for mcin in range(2):
                st = stg.tile([128, 4096], F32, tag="stg")
                stage.append(st)
            for n in range(8):
                for mcin in range(2):
                    p = ps.tile([128, 512], F32, tag="ps")
                    first = True
                    for (dy, dx) in taps:
                        sy, sx = dy - 1, dx - 1
                        for kc in range(2):
                            o, rhs = mm_views(fused[kc][:], p[:], sy, sx, n)
                            if o is None:
                                continue
                            lhsT = u1t[mcin][:, ((dy * 3 + dx) * 2 + kc) * 128 :
                                             ((dy * 3 + dx) * 2 + kc + 1) * 128]
                            nc.tensor.matmul(o, lhsT, rhs, start=first,
                                             stop=((dy, dx) == taps[-1] and kc == 1),
                                             skip_group_check=True)
                            first = False
                    nc.scalar.activation(stage[mcin][:, n * 512 : (n + 1) * 512], p[:],
                                         AF.Identity,
                                         bias=wt["up1b"][:, p4 * 2 + mcin : p4 * 2 + mcin + 1])
                p2 = ps.tile([128, 512], F32, tag="ps")
                for kc in range(2):
                    nc.tensor.matmul(p2[:], wt["up2w"][:, kc * 128 : (kc + 1) * 128],
                                     stage[kc][:, n * 512 : (n + 1) * 512],
                                     start=(kc == 0), stop=(kc == 1))
                nc.scalar.activation(pimg(up2p[p4][:])[:, n * 8 : (n + 1) * 8, 1:65],
                                     p2[:].rearrange("p (y x) -> p y x", y=8),
                                     AF.Identity, bias=wt["up2b"][:])

        # ---------------- re1 (polyphase 3x3, M-packed) ----------------
        re1t = big.tile([128, 4224], F32, tag="big")
        nc.gpsimd.memset(re1t[:], 0.0)
        for n in range(8):
            p = ps.tile([128, 512], F32, tag="ps")
            for ki, (pin, qy, qx) in enumerate(re1_keys):
                o, rhs = mm_views(up2p[pin][:], p[:], qy, qx, n)
                if o is None:
                    continue
                nc.tensor.matmul(o, wt["re1w"][:, ki * 128 : (ki + 1) * 128], rhs,
                                 start=(ki == 0), stop=(ki == len(re1_keys) - 1),
                                 skip_group_check=True)
            nc.scalar.activation(pimg(re1t[:])[:, n * 8 : (n + 1) * 8, 1:65],
                                 p[:].rearrange("p (y x) -> p y x", y=8),
                                 AF.Relu, bias=wt["re1b"][:])

        # ---------------- re2 (polyphase 3x3) + residual + interleave + out ----------------
        for n in range(8):
            pss = []
            for p4 in range(4):
                p = ps.tile([128, 512], F32, tag="ps")
                for qi, (qy, qx) in enumerate(re2_q[p4]):
                    o, rhs = mm_views(re1t[:], p[:], qy, qx, n)
                    if o is None:
                        continue
                    nc.tensor.matmul(o, wt["re2w"][:, (p4 * 4 + qi) * 128 :
                                                   (p4 * 4 + qi + 1) * 128], rhs,
                                     start=(qi == 0), stop=(qi == len(re2_q[p4]) - 1),
                                     skip_group_check=True)
                pss.append(p)
            for hb in range(2):  # half-bands of 8 output rows (4 phase rows)
                band = bnd.tile([128, 1024], F32, tag="bnd")
                bv = band[:].rearrange("p (y r x s) -> p y r x s", y=4, r=2, s=2)
                for p4 in range(4):
                    r, s = p4 // 2, p4 % 2
                    y0 = n * 8 + hb * 4
                    up_v = pimg(up2p[p4][:])[:, y0 : y0 + 4, 1:65]
                    nc.vector.scalar_tensor_tensor(
                        bv[:, :, r, :, s],
                        pss[p4][:, hb * 256 : (hb + 1) * 256].rearrange(
                            "p (y x) -> p y x", y=4),
                        wt["re2b"][:], up_v, AL.add, AL.add)
                nc.sync.dma_start(
                    out_d.ap()[:, (2 * n + hb) * 1024 : (2 * n + hb + 1) * 1024],
                    band[:])

    nc.compile()
    return nc


_NC = None


def _get_nc():
    global _NC
    if _NC is None:
        _NC = build()
    return _NC


def make_in_maps(inputs):
    w, _, _ = host_prep(inputs)
    x1 = np.ascontiguousarray(np.asarray(inputs["x1"], np.float32).reshape(NC, 256, 4096))
    x2 = np.ascontiguousarray(np.asarray(inputs["x2"], np.float32).reshape(NC, 256, 4096))
    in_maps = []
    for i in range(NC):
        m = {"x1": x1[i], "x2": x2[i]}
        m.update(w)
        in_maps.append(m)
    return in_maps


def kernel(**inputs):
    nc = _get_nc()
    in_maps = make_in_maps(inputs)
    res = run_bass_kernel_spmd(nc, in_maps, core_ids=list(range(NC)))
    out = np.stack([res.results[i]["out"].reshape(128, 128, 128) for i in range(NC)])
    return out.astype(np.float32)


# revision 20
# speedup vs baseline: 60.1013x; 1.0006x over previous
import sys

sys.path.insert(0, "/opt/trn_rl_repo")

from contextlib import ExitStack

import numpy as np

import concourse.bacc as bacc
import concourse.mybir as mybir
from concourse import tile
from concourse.bass_utils import run_bass_kernel_spmd

F32 = mybir.dt.float32
AL = mybir.AluOpType
AF = mybir.ActivationFunctionType

C = 256
H = W = 64
NC = 8  # cores / batch shards


# ---------------------------------------------------------------- host prep
def host_prep(inp):
    """Rearrange all weights into [partition, free] layouts matching SBUF tiles."""
    d = {}
    f = np.float32

    # conditioning nets (dsc1, dsc2)
    for i, pre in ((0, "dsc1"), (1, "dsc2")):
        w1 = np.asarray(inp[f"{pre}_w1"], f)  # [64, 256]
        b1 = np.asarray(inp[f"{pre}_b1"], f)  # [64]
        w2 = np.asarray(inp[f"{pre}_w2"], f)  # [2304, 64]
        b2 = np.asarray(inp[f"{pre}_b2"], f)  # [2304]
        # lhsT for gm matmul: [k_local, chunk, m]; fold the 1/(H*W) mean here
        d[f"w1T{i}"] = np.ascontiguousarray(
            (w1.T / (H * W)).reshape(2, 128, 64).transpose(1, 0, 2)
        ).reshape(128, 128)
        d[f"b1_{i}"] = b1.reshape(64, 1).copy()
        # lhsT for wts matmul: [j, chunk, k, c_local]; fold gelu's 0.5 here
        d[f"w2r{i}"] = np.ascontiguousarray(
            (0.5 * w2).reshape(2, 128, 9, 64).transpose(3, 0, 2, 1)
        ).reshape(64, 2304)
        d[f"b2r{i}"] = np.ascontiguousarray(
            b2.reshape(2, 128, 9).transpose(1, 0, 2)
        ).reshape(128, 18)

    # channel_align 1x1: [k_local, kc, mc, m]
    aw = np.asarray(inp["align_w"], f)[:, :, 0, 0]  # [256, 512]
    d["alignw"] = np.ascontiguousarray(
        aw.reshape(2, 128, 4, 128).transpose(3, 2, 0, 1)
    ).reshape(128, 1024)
    d["alignb"] = np.ascontiguousarray(
        np.asarray(inp["align_b"], f).reshape(2, 128).T
    )  # [128, 2]

    # up conv1 3x3 C->4C with pixel-shuffle phase reorder:
    # new channel (p, g) -> orig channel 4g + p   (p = 2r+s)
    uw = np.asarray(inp["up_w1"], f)  # [1024, 256, 3, 3]
    a = uw.reshape(256, 4, 2, 128, 9)  # [g, p, kc, k_local, tap]
    a = a.reshape(2, 128, 4, 2, 128, 9)  # [mcin, m, p, kc, k_local, tap]
    d["up1w"] = np.ascontiguousarray(a.transpose(4, 2, 0, 5, 3, 1)).reshape(
        128, 4 * 2 * 9 * 2 * 128
    )  # [k_local, (p, mcin, tap, kc, m)]
    ub = np.asarray(inp["up_b1"], f)
    d["up1b"] = np.ascontiguousarray(
        ub.reshape(2, 128, 4).transpose(1, 2, 0)
    ).reshape(128, 8)  # [m, (p, mcin)]

    # up conv2 1x1 C->C/2 (per-phase): [k_local, kc, m]
    u2 = np.asarray(inp["up_w2"], f)[:, :, 0, 0]  # [128, 256]
    d["up2w"] = np.ascontiguousarray(
        u2.reshape(128, 2, 128).transpose(2, 1, 0)
    ).reshape(128, 256)
    d["up2b"] = np.asarray(inp["up_b2"], f).reshape(128, 1).copy()

    # ---- polyphase re_enhance ----
    # phase/tap -> (in-phase, sub-shift) mapping
    def split(v):  # v = r + dy - 1
        rp = v % 2
        return rp, (v - rp) // 2

    r1w = np.asarray(inp["re_w1"], f)  # [32, 128, 3, 3]
    keymap = {}
    for p in range(4):
        r, s = p // 2, p % 2
        for dy in range(3):
            for dx in range(3):
                rp, qy = split(r + dy - 1)
                sp, qx = split(s + dx - 1)
                keymap.setdefault((2 * rp + sp, qy, qx), []).append((p, dy, dx))
    keys = sorted(keymap.keys(), key=lambda k: (k[1] != 0 or k[2] != 0, k))
    re1_keys = keys  # list of (p_in, qy, qx); all-(0,0) shifts first
    re1w = np.zeros((128, 16, 128), f)
    for ki, key in enumerate(keys):
        for (p, dy, dx) in keymap[key]:
            re1w[:, ki, p * 32 : (p + 1) * 32] = r1w[:, :, dy, dx].T
    d["re1w"] = re1w.reshape(128, 2048)
    d["re1b"] = np.tile(np.asarray(inp["re_b1"], f), 4).reshape(128, 1)

    r2w = np.asarray(inp["re_w2"], f)  # [128, 32, 3, 3]
    re2_q = []  # per out-phase list of shifts, (0,0) first
    re2w = np.zeros((128, 4, 4, 128), f)
    for p in range(4):
        r, s = p // 2, p % 2
        qys = sorted({split(r + dy - 1)[1] for dy in range(3)}, key=lambda q: q != 0)
        qxs = sorted({split(s + dx - 1)[1] for dx in range(3)}, key=lambda q: q != 0)
        qs = [(qy, qx) for qy in qys for qx in qxs]
        qs.sort(key=lambda q: q != (0, 0))
        re2_q.append(qs)
        for qi, (qy, qx) in enumerate(qs):
            for pp in range(4):
                rp, sp = pp // 2, pp % 2
                dy = 2 * qy + rp - r + 1
                dx = 2 * qx + sp - s + 1
                if 0 <= dy < 3 and 0 <= dx < 3:
                    re2w[pp * 32 : (pp + 1) * 32, p, qi, :] = r2w[:, :, dy, dx].T
    d["re2w"] = re2w.reshape(128, 2048)
    d["re2b"] = np.asarray(inp["re_b2"], f).reshape(128, 1).copy()

    return d, re1_keys, re2_q


RE1_KEYS = None
RE2_Q = None


def _mapping():
    global RE1_KEYS, RE2_Q
    if RE1_KEYS is None:
        zeros = {k: np.zeros(v) for k, v in [
            ("dsc1_w1", (64, 256)), ("dsc1_b1", (64,)), ("dsc1_w2", (2304, 64)),
            ("dsc1_b2", (2304,)), ("dsc2_w1", (64, 256)), ("dsc2_b1", (64,)),
            ("dsc2_w2", (2304, 64)), ("dsc2_b2", (2304,)),
            ("align_w", (256, 512, 1, 1)), ("align_b", (256,)),
            ("up_w1", (1024, 256, 3, 3)), ("up_b1", (1024,)),
            ("up_w2", (128, 256, 1, 1)), ("up_b2", (128,)),
            ("re_w1", (32, 128, 3, 3)), ("re_b1", (32,)),
            ("re_w2", (128, 32, 3, 3)), ("re_b2", (128,)),
        ]}
        _, RE1_KEYS, RE2_Q = host_prep(zeros)
    return RE1_KEYS, RE2_Q


# ---------------------------------------------------------------- bass build
def img(ap):
    return ap.rearrange("p (y x) -> p y x", y=64)


def shift_views(tile_ap, sy, sx):
    """(src, dst) views of a flat [128,4096] image for shift (sy,sx)."""
    y0 = max(0, -sy)
    r = 64 - abs(sy)
    x0 = max(0, -sx)
    xw = 64 - abs(sx)
    v = img(tile_ap)
    src = v[:, y0 + sy : y0 + sy + r, x0 + sx : x0 + sx + xw]
    dst = v[:, y0 : y0 + r, x0 : x0 + xw]
    return src, dst


def pimg(ap):
    """View of a column-padded [128, 64*66] image (zero cols at x=0 and x=65)."""
    return ap.rearrange("p (y x) -> p y x", x=66)


def mm_views(src_ap, psum_ap, sy, sx, n):
    """(psum_out, rhs) for 8-row chunk n of a shifted conv tap on a col-padded
    source. Row range restricted by sy; columns handled by the zero pad.
    psum out is a contiguous 2D region."""
    y0 = max(0, -sy)
    y1 = 64 + min(0, -sy)
    r0 = max(8 * n, y0)
    r1 = min(8 * n + 8, y1)
    if r1 <= r0:
        return None, None
    v = pimg(src_ap)
    rhs = v[:, r0 + sy : r1 + sy, 1 + sx : 65 + sx]
    out = psum_ap[:, (r0 - 8 * n) * 64 : (r1 - 8 * n) * 64]
    return out, rhs


def build():
    re1_keys, re2_q = _mapping()
    nc = bacc.Bacc(trn_type="TRN2", target_bir_lowering=False, debug=False)

    x_d = [nc.dram_tensor(n, [256, 4096], F32, kind="ExternalInput") for n in ("x1", "x2")]
    wd = {}
    for name, shape in [
        ("w1T0", [128, 128]), ("w1T1", [128, 128]),
        ("b1_0", [64, 1]), ("b1_1", [64, 1]),
        ("w2r0", [64, 2304]), ("w2r1", [64, 2304]),
        ("b2r0", [128, 18]), ("b2r1", [128, 18]),
        ("alignw", [128, 1024]), ("alignb", [128, 2]),
        ("up1w", [128, 18432]), ("up1b", [128, 8]),
        ("up2w", [128, 256]), ("up2b", [128, 1]),
        ("re1w", [128, 2048]), ("re1b", [128, 1]),
        ("re2w", [128, 2048]), ("re2b", [128, 1]),
    ]:
        wd[name] = nc.dram_tensor(name, shape, F32, kind="ExternalInput")
    out_d = nc.dram_tensor("out", [128, 16384], F32, kind="ExternalOutput")

    with tile.TileContext(nc) as tc, ExitStack() as ctx:
        wpool = ctx.enter_context(tc.tile_pool(name="w", bufs=1))
        u1pool = ctx.enter_context(tc.tile_pool(name="u1w", bufs=2))
        big = ctx.enter_context(tc.tile_pool(name="big", bufs=7))
        stg = ctx.enter_context(tc.tile_pool(name="stg", bufs=2))
        bnd = ctx.enter_context(tc.tile_pool(name="bnd", bufs=2))
        tiny = ctx.enter_context(tc.tile_pool(name="tiny", bufs=6))
        ps = ctx.enter_context(tc.tile_pool(name="ps", bufs=6, space="PSUM"))
        psc = ctx.enter_context(tc.tile_pool(name="psc", bufs=1, space="PSUM"))

        # persistent small weights
        wt = {}
        for name in ("w1T0", "w1T1", "b1_0", "b1_1", "b2r0", "b2r1",
                     "alignw", "alignb", "up2w", "up2b",
                     "re1w", "re1b", "re2w", "re2b", "up1b"):
            t = wpool.tile(list(wd[name].shape), F32, tag=name)
            nc.sync.dma_start(t[:], wd[name].ap())
            wt[name] = t

        # x in
        xin = []  # [input][chunk] -> big tile
        for i in range(2):
            pair = []
            for c in range(2):
                t = big.tile([128, 4096], F32, tag="big")
                nc.sync.dma_start(t[:], x_d[i].ap()[c * 128 : (c + 1) * 128, :])
                pair.append(t)
            xin.append(pair)

        # streamed conditioning w2r (one dsc at a time)
        w2rt = {}

        def load_w2r(d):
            if d not in w2rt:
                t = u1pool.tile([64, 2304], F32, tag="u1w")
                nc.sync.dma_start(t[:], wd[f"w2r{d}"].ap())
                w2rt[d] = t
            return w2rt[d]

        # ---------------- dsc stage ----------------
        def conditioning(d, gms):
            """gms: list of two [128,1] sum tiles -> wts [chunk][128,9], w4p [chunk][128,1]."""
            pg = psc.tile([64, 1], F32, tag="psc_g")
            for c in range(2):
                nc.tensor.matmul(pg[:], wt[f"w1T{d}"][:, c * 64 : (c + 1) * 64],
                                 gms[c][:], start=(c == 0), stop=(c == 1))
            u = tiny.tile([64, 1], F32, tag="u")
            nc.scalar.activation(u[:], pg[:], AF.Identity, bias=wt[f"b1_{d}"][:])
            sq = tiny.tile([64, 1], F32, tag="sq")
            nc.scalar.activation(sq[:], u[:], AF.Square)
            v3 = tiny.tile([64, 1], F32, tag="v3")
            nc.vector.scalar_tensor_tensor(v3[:], sq[:], 0.044715, u[:], AL.mult, AL.mult)
            w_ = tiny.tile([64, 1], F32, tag="w_")
            nc.vector.tensor_tensor(w_[:], v3[:], u[:], AL.add)
            th = tiny.tile([64, 1], F32, tag="th")
            nc.scalar.activation(th[:], w_[:], AF.Tanh, scale=0.7978845608028654)
            hv = tiny.tile([64, 1], F32, tag="hv")
            nc.vector.scalar_tensor_tensor(hv[:], th[:], 1.0, u[:], AL.add, AL.mult)

            w2r = load_w2r(d)
            wts_l, w4p_l = [], []
            for c in range(2):
                pw = psc.tile([128, 9], F32, tag="psc_w")
                for k in range(9):
                    nc.tensor.matmul(pw[:, k : k + 1],
                                     w2r[:, (c * 9 + k) * 128 : (c * 9 + k + 1) * 128],
                                     hv[:], start=True, stop=True)
                raw = tiny.tile([128, 9], F32, tag="raw")
                for k in range(9):
                    nc.scalar.activation(raw[:, k : k + 1], pw[:, k : k + 1], AF.Identity,
                                         bias=wt[f"b2r{d}"][:, c * 9 + k : c * 9 + k + 1])
                mx = tiny.tile([128, 1], F32, tag="mx")
                nc.vector.tensor_reduce(mx[:], raw[:], axis=mybir.AxisListType.X, op=AL.max)
                ngm = tiny.tile([128, 1], F32, tag="ngm")
                nc.vector.tensor_scalar_mul(ngm[:], mx[:], -1.0)
                ex = tiny.tile([128, 9], F32, tag="ex")
                ssum = tiny.tile([128, 1], F32, tag="ssum")
                nc.scalar.activation(ex[:], raw[:], AF.Exp, bias=ngm[:], accum_out=ssum[:])
                rec = tiny.tile([128, 1], F32, tag="rec")
                nc.vector.reciprocal(rec[:], ssum[:])
                wts = tiny.tile([128, 9], F32, tag="wts")
                nc.vector.tensor_scalar_mul(wts[:], ex[:], rec[:])
                w4p = tiny.tile([128, 1], F32, tag="w4p")
                nc.vector.tensor_scalar_add(w4p[:], wts[:, 4:5], 1.0)
                wts_l.append(wts)
                w4p_l.append(w4p)
            return wts_l, w4p_l

        def depthwise(src, dst, wts, w4p):
            """dst = sum_k wts[:,k] * shift_k(src) + src   (flat [128,4096] tiles)."""
            nc.vector.tensor_scalar(dst[:], src[:], w4p[:], None, AL.mult)
            for k in (0, 1, 2, 3, 5, 6, 7, 8):
                sy, sx = k // 3 - 1, k % 3 - 1
                sv, dv = shift_views(src[:], sy, sx)
                _, acc = shift_views(dst[:], sy, sx)
                nc.vector.scalar_tensor_tensor(acc, sv, wts[:, k : k + 1], acc,
                                               AL.mult, AL.add)

        y = [[None, None], [None, None]]  # dyn_block outputs
        mid = [[None, None], [None, None]]
        gms2 = [[None, None], [None, None]]
        for i in range(2):
            gms1 = []
            for c in range(2):
                g = tiny.tile([128, 1], F32, tag="gms")
                nc.vector.tensor_reduce(g[:], xin[i][c][:], axis=mybir.AxisListType.X, op=AL.add)
                gms1.append(g)
            wts1, w4p1 = conditioning(0, gms1)
            for c in range(2):
                m = big.tile([128, 4096], F32, tag="big")
                depthwise(xin[i][c], m, wts1[c], w4p1[c])
                g2 = tiny.tile([128, 1], F32, tag="gms")
                nc.scalar.activation(m[:], m[:], AF.Relu, accum_out=g2[:])
                mid[i][c] = m
                gms2[i][c] = g2
        for i in range(2):
            wts2, w4p2 = conditioning(1, gms2[i])
            for c in range(2):
                yt = big.tile([128, 4096], F32, tag="big")
                depthwise(mid[i][c], yt, wts2[c], w4p2[c])
                y[i][c] = yt

        # ---------------- align 1x1 (2C -> C) ----------------
        fused = []
        for mc in range(2):
            f = big.tile([128, 4224], F32, tag="big")
            nc.gpsimd.memset(f[:], 0.0)
            fused.append(f)
        for mc in range(2):
            for n in range(8):
                p = ps.tile([128, 512], F32, tag="ps")
                for kc in range(4):
                    rhs = y[kc // 2][kc % 2][:, n * 512 : (n + 1) * 512]
                    nc.tensor.matmul(
                        p[:], wt["alignw"][:, (kc * 2 + mc) * 128 : (kc * 2 + mc + 1) * 128],
                        rhs, start=(kc == 0), stop=(kc == 3))
                nc.scalar.activation(pimg(fused[mc][:])[:, n * 8 : (n + 1) * 8, 1:65],
                                     p[:].rearrange("p (y x) -> p y x", y=8),
                                     AF.Identity, bias=wt["alignb"][:, mc : mc + 1])

        # ---------------- up1 (3x3 C->4C, phase-ordered) + up2 (1x1) ----------------
        taps = [(1, 1)] + [(dy, dx) for dy in range(3) for dx in range(3) if (dy, dx) != (1, 1)]
        up2p = []
        for p4 in range(4):
            t = big.tile([128, 4224], F32, tag="big")
            nc.gpsimd.memset(t[:], 0.0)
            up2p.append(t)
        for p4 in range(4):
            u1t = []
            for mcin in range(2):
                t = u1pool.tile([128, 2304], F32, tag="u1w")
                off = (p4 * 2 + mcin) * 2304
                nc.sync.dma_start(t[:], wd["up1w"].ap()[:, off : off + 2304])
                u1t.append(t)
            stage = []
            for mcin in range(2):
                st = stg.tile([128, 4096], F32, tag="stg")
                stage.append(st)
            for n in range(8):
                for mcin in range(2):
                    p = ps.tile([128, 512], F32, tag="ps")
                    first = True
                    for (dy, dx) in taps:
                        sy, sx = dy - 1, dx - 1
                        for kc in range(2):
                            o, rhs = mm_views(fused[kc][:], p[:], sy, sx, n)
                            if o is None:
                                continue
                            lhsT = u1t[mcin][:, ((dy * 3 + dx) * 2 + kc) * 128 :
                                             ((dy * 3 + dx) * 2 + kc + 1) * 128]
                            nc.tensor.matmul(o, lhsT, rhs, start=first,
                                             stop=((dy, dx) == taps[-1] and kc == 1),
                                             skip_group_check=True)
                            first = False
                    nc.scalar.activation(stage[mcin][:, n * 512 : (n + 1) * 512], p[:],
                                         AF.Identity,
                                         bias=wt["up1b"][:, p4 * 2 + mcin : p4 * 2 + mcin + 1])
                p2 = ps.tile([128, 512], F32, tag="ps")
                for kc in range(2):
                    nc.tensor.matmul(p2[:], wt["up2w"][:, kc * 128 : (kc + 1) * 128],
                                     stage[kc][:, n * 512 : (n + 1) * 512],
                                     start=(kc == 0), stop=(kc == 1))
                nc.scalar.activation(pimg(up2p[p4][:])[:, n * 8 : (n + 1) * 8, 1:65],
                                     p2[:].rearrange("p (y x) -> p y x", y=8),
                                     AF.Identity, bias=wt["up2b"][:])

        # ---------------- re1 (polyphase 3x3, M-packed) ----------------
        re1t = big.tile([128, 4224], F32, tag="big")
        nc.gpsimd.memset(re1t[:], 0.0)
        for n in range(8):
            p = ps.tile([128, 512], F32, tag="ps")
            for ki, (pin, qy, qx) in enumerate(re1_keys):
                o, rhs = mm_views(up2p[pin][:], p[:], qy, qx, n)
                if o is None:
                    continue
                nc.tensor.matmul(o, wt["re1w"][:, ki * 128 : (ki + 1) * 128], rhs,
                                 start=(ki == 0), stop=(ki == len(re1_keys) - 1),
                                 skip_group_check=True)
            nc.scalar.activation(pimg(re1t[:])[:, n * 8 : (n + 1) * 8, 1:65],
                                 p[:].rearrange("p (y x) -> p y x", y=8),
                                 AF.Relu, bias=wt["re1b"][:])

        # ---------------- re2 (polyphase 3x3) + residual + interleave + out ----------------
        for n in range(8):
            pss = []
            for p4 in range(4):
                p = ps.tile([128, 512], F32, tag="ps")
                for qi, (qy, qx) in enumerate(re2_q[p4]):
                    o, rhs = mm_views(re1t[:], p[:], qy, qx, n)
                    if o is None:
                        continue
                    nc.tensor.matmul(o, wt["re2w"][:, (p4 * 4 + qi) * 128 :
                                                   (p4 * 4 + qi + 1) * 128], rhs,
                                     start=(qi == 0), stop=(qi == len(re2_q[p4]) - 1),
                                     skip_group_check=True)
                pss.append(p)
            for hb in range(2):  # half-bands of 8 output rows (4 phase rows)
                band = bnd.tile([128, 1024], F32, tag="bnd")
                bv = band[:].rearrange("p (y r x s) -> p y r x s", y=4, r=2, s=2)
                for p4 in range(4):
                    r, s = p4 // 2, p4 % 2
                    y0 = n * 8 + hb * 4
                    up_v = pimg(up2p[p4][:])[:, y0 : y0 + 4, 1:65]
                    nc.vector.scalar_tensor_tensor(
                        bv[:, :, r, :, s],
                        pss[p4][:, hb * 256 : (hb + 1) * 256].rearrange(
                            "p (y x) -> p y x", y=4),
                        wt["re2b"][:], up_v, AL.add, AL.add)
                nc.sync.dma_start(
                    out_d.ap()[:, (2 * n + hb) * 1024 : (2 * n + hb + 1) * 1024],
                    band[:])

    nc.compile()
    return nc


_NC = None


def _get_nc():
    global _NC
    if _NC is None:
        _NC = build()
    return _NC


def make_in_maps(inputs):
    w, _, _ = host_prep(inputs)
    x1 = np.ascontiguousarray(np.asarray(inputs["x1"], np.float32).reshape(NC, 256, 4096))
    x2 = np.ascontiguousarray(np.asarray(inputs["x2"], np.float32).reshape(NC, 256, 4096))
    in_maps = []
    for i in range(NC):
        m = {"x1": x1[i], "x2": x2[i]}
        m.update(w)
        in_maps.append(m)
    return in_maps


def kernel(**inputs):
    nc = _get_nc()
    in_maps = make_in_maps(inputs)
    res = run_bass_kernel_spmd(nc, in_maps, core_ids=list(range(NC)))
    out = np.stack([res.results[i]["out"].reshape(128, 128, 128) for i in range(NC)])
    return out.astype(np.float32)
